# revision 24
# baseline (speedup 1.0000x reference)
"""Trainium2 Bass kernel for CdfgReader GNN message passing (fp16/fp8).

Strategy:
  - 64 batch items draw from <=32 unique CDFGs: compute the GNN once per
    unique graph; ceil(u/8) graph slots per core across 8 cores (SPMD).
  - Numerics (tolerance 2e-2, ~4e-3 measured end-to-end):
    * Main datapath fp16: activations x, A (binary, exact in any fp dtype),
      XW operands.  PE cost for fp16 == f32r (1 cycle/row) but DMA/SBUF
      halve and the vector engines work on half the bytes.
    * Weight systematic rounding error (the error-budget driver) is killed
      by a cheap fp8 DoubleRow correction matmul per X@W:
      XW = x16 @ fp16(W)*2^c + x8 @ e5m2((W - fp16(W)) * 2^(s+c)), where
      x8 = e4m3(x / 2^s).  DR contracts 2x128 rows at 0.5 cycles/row so the
      correction costs 1/4 of a full lo-pass.
    * Per-layer power-of-2 "lifts" 2^c are folded into the weights so every
      fp8 tensor sits in e4m3/e5m2 range; the relu/tanh ACT descales for
      free via its scale operand.
    * A-multiply per hidden layer is configurable: "f16" (dense fp16), "2t"
      (XW split hi+mid e4m3, DR at 0.5 cyc/row, 2x fewer PE cycles), "3t"
      (hi+mid+lo, ~11 effective bits, 1.33x fewer cycles).  The final layer
      always runs hi+lo DR fp8, restricted to the K 128-node tiles holding
      the coverpoint-mask union (host permutes nodes so the union is first).
    * XW psums are evicted to SBUF fp16 by one DVE copy; the fp8 splits
      derive from that copy so PSUM banks recycle at PE pace.
  - Masked mean via small fp16 mask matmuls (columns carry 1/count)
    accumulated in a dedicated PSUM bank across all slots.
"""

import numpy as np

NCORES = 8
N = 1024        # max nodes
F = 128         # input feature dim
H = 256         # hidden dim
L = 4           # GCN layers
B = 64          # batch (coverpoints)

# per-layer A-mult mode for hidden layers 0..2: "f16" | "2t" | "3t"
AMODE = ("f16", "f16", "2t")

C_IN = 4                 # lift on W_in
C_L = (5, 3, 1, -1)      # lift on W_gcn[l]
S_X = (0, 0, 5)          # x0/x1/x2 fp8 copy scales (x2's feeds XW3 corr too)
S_XS = 2                 # xs fp8 copy scale

_CACHE = {}


def _build_nc(NG, Ks):
    import concourse.bass as bass  # noqa: F401
    import concourse.mybir as mybir
    import concourse.tile as tile
    from concourse import bacc
    from concourse.bass import ts

    f32 = mybir.dt.float32
    f16 = mybir.dt.float16
    e4 = mybir.dt.float8e4
    e5 = mybir.dt.float8e5
    DR = mybir.MatmulPerfMode.DoubleRow
    Relu = mybir.ActivationFunctionType.Relu
    Tanh = mybir.ActivationFunctionType.Tanh
    Copy = mybir.ActivationFunctionType.Copy
    subop = mybir.AluOpType.subtract

    T = sum(Ks)
    offs = np.concatenate([[0], np.cumsum(Ks)]).astype(int)
    Kmax = max(Ks)
    need_a16 = False  # f16 A-mult reads the exact fp8 A (binary)

    nc = bacc.Bacc("TRN2", target_bir_lowering=False, debug=False,
                   num_devices=NCORES)

    a16_t = (nc.dram_tensor("a16_t", [NG, N, N], f16, kind="ExternalInput")
             if need_a16 else None)
    a8_t = nc.dram_tensor("a8_t", [NG, N, N], e4, kind="ExternalInput")
    xs16_t = nc.dram_tensor("xs16_t", [F, NG, N], f16, kind="ExternalInput")
    xs8d_t = nc.dram_tensor("xs8d_t", [F, NG, 2, N], e4, kind="ExternalInput")
    m16_t = nc.dram_tensor("m16_t", [128, T, B], f16, kind="ExternalInput")
    win16_t = nc.dram_tensor("win16_t", [128, H], f16, kind="ExternalInput")
    dwin8_t = nc.dram_tensor("dwin8_t", [128, 2, H], e5, kind="ExternalInput")
    wg16_t = nc.dram_tensor("wg16_t", [128, 2 * L, H], f16,
                            kind="ExternalInput")
    dwg8_t = nc.dram_tensor("dwg8_t", [128, 2 * L, H], e5,
                            kind="ExternalInput")
    out = nc.dram_tensor("out", [B, H], f32, kind="ExternalOutput")

    with tile.TileContext(nc) as tc:
        with (
            tc.tile_pool(name="const", bufs=1) as constp,
            tc.tile_pool(name="a16p", bufs=2) as a16p,
            tc.tile_pool(name="a8p", bufs=2) as a8p,
            tc.tile_pool(name="xsp", bufs=2) as xsp_pool,
            tc.tile_pool(name="xs8p", bufs=2) as xs8p,
            tc.tile_pool(name="x0p", bufs=2) as x0p,
            tc.tile_pool(name="x08p", bufs=2) as x08p,
            tc.tile_pool(name="x0np", bufs=2) as x0np,
            tc.tile_pool(name="xp", bufs=2) as xp,
            tc.tile_pool(name="x8p", bufs=2) as x8pool,
            tc.tile_pool(name="xwp", bufs=2) as xwp,
            tc.tile_pool(name="xw8p", bufs=2) as xw8p,
            tc.tile_pool(name="xfp", bufs=2) as xfp,
            tc.tile_pool(name="psx", bufs=3, space="PSUM") as psx,
            tc.tile_pool(name="psw", bufs=3, space="PSUM") as psw,
            tc.tile_pool(name="pacc", bufs=1, space="PSUM") as paccp,
        ):
            # ---- DMA priority: input-layer operands first
            xsp0 = xsp_pool.tile([128, 2, N], f16, tag="xs", name="xs_p")
            nc.sync.dma_start(xsp0[:, 0, 0:512], xs16_t[:, 0, 0:512])
            win16_sb = constp.tile([128, H], f16)
            nc.sync.dma_start(win16_sb[:], win16_t[:, :])
            dwin8_sb = constp.tile([128, 2, H], e5)
            nc.sync.dma_start(dwin8_sb[:], dwin8_t[:, :, :])
            xs8d0 = xs8p.tile([128, 2, N], e4, tag="xs8", name="xs8_p")
            nc.sync.dma_start(xs8d0[:], xs8d_t[:, 0, :, :])
            nc.sync.dma_start(xsp0[:, 0, 512:1024], xs16_t[:, 0, 512:1024])
            if NG > 1:
                nc.sync.dma_start(xsp0[:, 1, :], xs16_t[:, 1, :])
            xs8d1 = None
            if NG > 1:
                xs8d1 = xs8p.tile([128, 2, N], e4, tag="xs8", name="xs8_p")
                nc.sync.dma_start(xs8d1[:], xs8d_t[:, 1, :, :])

            wg16_sb = constp.tile([128, 2 * L, H], f16)
            dwg8_sb = constp.tile([128, 2 * L, H], e5)
            # layer-0 weights first so slot 0's layer 0 starts at DMA pace
            nc.sync.dma_start(wg16_sb[:, 0:2, :], wg16_t[:, 0:2, :])
            nc.sync.dma_start(dwg8_sb[:, 0:2, :], dwg8_t[:, 0:2, :])

            a16_sb0 = None
            if need_a16:
                a16_sb0 = a16p.tile([128, 8, N], f16, tag="a16", name="a16_sb")
                for q in range(4):
                    nc.sync.dma_start(
                        a16_sb0[:, 2 * q:2 * q + 2, :],
                        a16_t[0, 256 * q:256 * q + 256, :]
                        .rearrange("(mo p) i -> p mo i", p=128))
            a8_sb0 = a8p.tile([128, 8, N], e4, tag="a8", name="a8_sb")
            for q in range(4):
                nc.sync.dma_start(
                    a8_sb0[:, 2 * q:2 * q + 2, :],
                    a8_t[0, 256 * q:256 * q + 256, :]
                    .rearrange("(mo p) i -> p mo i", p=128))

            nc.sync.dma_start(wg16_sb[:, 2:8, :], wg16_t[:, 2:8, :])
            nc.sync.dma_start(dwg8_sb[:, 2:8, :], dwg8_t[:, 2:8, :])
            m16_sb = constp.tile([128, T, B], f16)
            nc.sync.dma_start(m16_sb[:], m16_t[:, :, :])

            out_sb = constp.tile([B, H], f32)
            pacc = paccp.tile([128, H], f32, tag="pacc", name="pacc")
            n_pmc = [0]
            total_pmc = 2 * T

            def make_head(g, xs16_g, xs8d_g, off_g):
                x0t = x0p.tile([128, 2, N], f16, tag="x0", name="x0t")
                x08 = x08p.tile([128, 2, N], e4, tag="x08", name="x08")
                x0n = x0np.tile([128, Kmax, H], f16, tag="x0n", name="x0n")

                def x0t_group(i):
                    t, c = [(0, 0), (1, 0), (0, 1), (1, 1)][i]
                    ps = psx.tile([128, 512], f32, tag="psx", name="ps0")
                    nc.tensor.matmul(ps[:], win16_sb[:, ts(t, 128)],
                                     xs16_g[:, ts(c, 512)],
                                     start=True, stop=False)
                    nc.tensor.matmul(ps[:], dwin8_sb[:, :, ts(t, 128)],
                                     xs8d_g[:, :, ts(c, 512)],
                                     start=False, stop=True, perf_mode=DR)
                    nc.scalar.activation(x0t[:, t, ts(c, 512)], ps[:],
                                         Relu, scale=2.0 ** -C_IN)
                    # s_x0 == 0: plain dtype-cast copy on Pool
                    nc.gpsimd.tensor_copy(x08[:, t, ts(c, 512)],
                                          x0t[:, t, ts(c, 512)])

                def x0n_group(c, off_g=None):
                    ps = psw.tile([128, H], f32, tag="ps3", name="ps0n",
                                  bufs=2)
                    nc.tensor.matmul(ps[:], xs16_g[:, ts(c, 128)],
                                     win16_sb[:], start=True, stop=False)
                    nc.tensor.matmul(ps[:], xs8d_g[:, :, ts(c, 128)],
                                     dwin8_sb[:], start=False, stop=True,
                                     perf_mode=DR)
                    nc.scalar.activation(x0n[:, c, :], ps[:], Relu,
                                         scale=2.0 ** -C_IN)
                    # residual side of the masked mean: out += m^T x0n,
                    # accumulated into pacc independently of the l3 tiles
                    i = n_pmc[0]
                    n_pmc[0] += 1
                    nc.tensor.matmul(pacc[0:B, :],
                                     m16_sb[:, off_g + c, :],
                                     x0n[:, c, :],
                                     start=(i == 0),
                                     stop=(i == total_pmc - 1),
                                     skip_group_check=True)

                def x0n_group_o(c):
                    x0n_group(c, off_g)

                return {"x0t": x0t, "x08": x08, "x0n": x0n,
                        "x0t_group": x0t_group, "x0n_group": x0n_group_o}

            heads = {}
            slot_xs = {0: (xsp0[:, 0, :], xs8d0)}
            if NG > 1:
                slot_xs[1] = (xsp0[:, 1, :], xs8d1)

            for g in range(NG):
                K = Ks[g]
                off = int(offs[g])
                if g == 0:
                    a16_sb, a8_sb = a16_sb0, a8_sb0
                else:
                    a16_sb, a8_sb = slot_a[g]
                xs_g, xs8d_g = slot_xs[g]

                # prefetch next slot's tensors now (pools are double
                # buffered) so its head can run as l3-phase filler
                if g + 1 < NG:
                    gn = g + 1
                    if gn >= 2:
                        xsn = xsp_pool.tile([128, 2, N], f16, tag="xs",
                                            name="xs_p")
                        nc.sync.dma_start(xsn[:, 0, :], xs16_t[:, gn, :])
                        xs8dn = xs8p.tile([128, 2, N], e4, tag="xs8",
                                          name="xs8_p")
                        nc.sync.dma_start(xs8dn[:], xs8d_t[:, gn, :, :])
                        slot_xs[gn] = (xsn[:, 0, :], xs8dn)
                    a16_n = None
                    if need_a16:
                        a16_n = a16p.tile([128, 8, N], f16, tag="a16",
                                          name="a16_sb")
                        nc.sync.dma_start(
                            a16_n[:],
                            a16_t[gn].rearrange("(mo p) i -> p mo i", p=128))
                    a8_n = a8p.tile([128, 8, N], e4, tag="a8", name="a8_sb")
                    nc.sync.dma_start(
                        a8_n[:],
                        a8_t[gn].rearrange("(mo p) i -> p mo i", p=128))
                    if g == 0:
                        slot_a = {}
                    slot_a[gn] = (a16_n, a8_n)

                cs = list(range(K))
                if g in heads:
                    # x0t groups were emitted as l3 filler in the previous
                    # slot; all x0n groups remain as boundary fillers
                    hd = heads.pop(g)
                    fillq = [lambda c=c: hd["x0n_group"](c) for c in cs]
                else:
                    hd = make_head(g, xs_g, xs8d_g, off)
                    for i in range(4):
                        hd["x0t_group"](i)
                    nfill = min(3, K - 1)
                    for c in cs[:K - nfill]:
                        hd["x0n_group"](c)
                    fillq = [lambda c=c: hd["x0n_group"](c)
                             for c in cs[K - nfill:]]
                x0t, x08, x0n = hd["x0t"], hd["x08"], hd["x0n"]
                x0n_group = hd["x0n_group"]

                def fill():
                    if fillq:
                        fillq.pop(0)()

                def mk_stage(idx, x16, x8):
                    # idx 0..2: hidden-layer XW; idx 3: final XW3 (hi/lo)
                    mode = "xw3" if idx == 3 else AMODE[idx]
                    st = {"mode": mode}
                    if mode == "f16":
                        st["r16"] = xwp.tile([128, 8, H], f16, tag="xw",
                                             name="r16")
                    else:
                        st["xwh"] = xw8p.tile([128, 8, H], e4, tag="xwh",
                                              name="xwh")
                        st["xwm"] = xw8p.tile([128, 8, H], e4, tag="xwm",
                                              name="xwm")
                        if mode == "3t":
                            st["xwl"] = xw8p.tile([128, 8, H], e4, tag="xwl",
                                                  name="xwl")
                            st["r2"] = xwp.tile([128, 8, H], f16, tag="xw",
                                                name="r2")

                    def pair(p, idx=idx, mode=mode, st=st, x16=x16, x8=x8):
                        ps = psw.tile([128, 2, H], f32, tag="psw",
                                      name="psw", bufs=2)
                        for i in range(2):
                            ch = 2 * p + i
                            nc.tensor.matmul(ps[:, i, :],
                                             x16[:, 0, ts(ch, 128)],
                                             wg16_sb[:, 2 * idx, :],
                                             start=True, stop=False)
                            nc.tensor.matmul(ps[:, i, :],
                                             x16[:, 1, ts(ch, 128)],
                                             wg16_sb[:, 2 * idx + 1, :],
                                             start=False, stop=False)
                            nc.tensor.matmul(
                                ps[:, i, :], x8[:, :, ts(ch, 128)],
                                dwg8_sb[:, 2 * idx:2 * idx + 2, :],
                                start=False, stop=True, perf_mode=DR)
                        sl = np.s_[:, 2 * p:2 * p + 2, :]
                        if mode == "f16":
                            nc.vector.tensor_copy(st["r16"][sl], ps[:])
                        elif mode in ("2t", "xw3"):
                            nc.scalar.activation(st["xwh"][sl], ps[:], Copy)
                            nc.vector.tensor_tensor(st["xwm"][sl], ps[:],
                                                    st["xwh"][sl], subop)
                        else:
                            nc.scalar.activation(st["xwh"][sl], ps[:], Copy)
                            nc.vector.tensor_tensor(st["r2"][sl], ps[:],
                                                    st["xwh"][sl], subop)
                            nc.gpsimd.tensor_copy(st["xwm"][sl],
                                                  st["r2"][sl])
                            nc.vector.tensor_tensor(st["xwl"][sl],
                                                    st["r2"][sl],
                                                    st["xwm"][sl], subop)

                    st["pair"] = pair
                    return st

                st = mk_stage(0, x0t, x08)
                st["pair"](0)
                fill()
                st["pair"](1)
                st["pair"](2)
                fill()
                st["pair"](3)

                for layer in range(L - 1):
                    mode = AMODE[layer]
                    if layer > 0:
                        st["pair"](2)
                        fill()
                        st["pair"](3)

                    pssA = [psx.tile([128, 512], f32, tag="psx",
                                     name=f"psA{t_}") for t_ in range(2)]
                    pssB = [psx.tile([128, 512], f32, tag="psx",
                                     name=f"psB{t_}") for t_ in range(2)]

                    def a16_pass(pss, c, m, st=st, a8_sb=a8_sb):
                        for t in range(2):
                            nc.tensor.matmul(
                                pss[t][:], st["r16"][:, m, ts(t, 128)],
                                a8_sb[:, m, ts(c, 512)],
                                start=(m == 0), stop=(m == 7))

                    def a8_sweep(pss, c, term, first, last, a8_sb=a8_sb,
                                 prange=(0, 4)):
                        for p in range(*prange):
                            for t in range(2):
                                nc.tensor.matmul(
                                    pss[t][:],
                                    term[:, 2 * p:2 * p + 2, ts(t, 128)],
                                    a8_sb[:, 2 * p:2 * p + 2, ts(c, 512)],
                                    start=(first and p == 0),
                                    stop=(last and p == 3),
                                    perf_mode=DR)

                    xn16 = xp.tile([128, 2, N], f16, tag="xn", name="xn16")
                    xn8 = x8pool.tile([128, 2, N], e4, tag="xn8", name="xn8")
                    s_next = S_X[min(layer + 1, 2)]

                    def act_half(pss, c, layer=layer, xn16=xn16, xn8=xn8,
                                 s_next=s_next):
                        nc.scalar.activation(xn16[:, 0, ts(c, 512)],
                                             pss[0][:], Relu,
                                             scale=2.0 ** -C_L[layer])
                        nc.vector.tensor_scalar(xn16[:, 1, ts(c, 512)],
                                                pss[1][:],
                                                2.0 ** -C_L[layer], 0.0,
                                                mybir.AluOpType.mult,
                                                mybir.AluOpType.max)
                        for t in range(2):
                            if s_next == 0:
                                nc.gpsimd.tensor_copy(
                                    xn8[:, t, ts(c, 512)],
                                    xn16[:, t, ts(c, 512)])
                            else:
                                nc.gpsimd.tensor_scalar_mul(
                                    xn8[:, t, ts(c, 512)],
                                    xn16[:, t, ts(c, 512)],
                                    2.0 ** -s_next)

                    # software pipeline: the next stage's first two XW pairs
                    # (which only need the c0 half) are emitted between the
                    # two act halves so every engine chain is PE-covered
                    nidx = layer + 1 if layer < 2 else 3
                    if mode == "f16":
                        for m in range(4):
                            a16_pass(pssA, 0, m)
                        fill()
                        for m in range(4):
                            a16_pass(pssB, 1, m)
                        for m in range(4, 8):
                            a16_pass(pssA, 0, m)
                        act_half(pssA, 0)
                        for m in range(4, 8):
                            a16_pass(pssB, 1, m)
                        nst = mk_stage(nidx, xn16, xn8)
                        nst["pair"](0)
                        nst["pair"](1)
                        act_half(pssB, 1)
                        fill()
                    else:
                        terms = ([st["xwh"], st["xwm"]] if mode == "2t"
                                 else [st["xwh"], st["xwm"], st["xwl"]])
                        nt = len(terms)
                        # first term's sweeps consume pairs 0-1 on both
                        # halves before touching pairs 2-3, buying the late
                        # pairs' ACT/DVE conversions more time
                        a8_sweep(pssA, 0, terms[0], True, nt == 1,
                                 prange=(0, 2))
                        fill()
                        a8_sweep(pssB, 1, terms[0], True, nt == 1,
                                 prange=(0, 2))
                        a8_sweep(pssA, 0, terms[0], False, nt == 1,
                                 prange=(2, 4))
                        fill()
                        a8_sweep(pssB, 1, terms[0], False, nt == 1,
                                 prange=(2, 4))
                        for j, term in enumerate(terms[1:-1], start=1):
                            a8_sweep(pssA, 0, term, False, False)
                            fill()
                            a8_sweep(pssB, 1, term, False, False)
                        if nt > 1:
                            a8_sweep(pssA, 0, terms[-1], False, True)
                        act_half(pssA, 0)
                        if nt > 1:
                            a8_sweep(pssB, 1, terms[-1], False, True)
                        nst = mk_stage(nidx, xn16, xn8)
                        nst["pair"](0)
                        nst["pair"](1)
                        act_half(pssB, 1)
                        fill()
                    st = nst

                # ---- final layer: DR hi/lo A-mult on the K masked tiles
                st["pair"](2)
                while fillq:
                    fillq.pop(0)()
                st["pair"](3)
                xw3h, xw3l = st["xwh"], st["xwm"]

                xf = xfp.tile([128, Kmax, H], f16, tag="xf", name="xf")

                def l3_tile(c, a8_sb=a8_sb, x0n=x0n, off=off):
                    ps = psw.tile([128, H], f32, tag="ps3", name="ps3",
                                  bufs=2)
                    for j, xt in enumerate((xw3h, xw3l)):
                        for p in range(4):
                            nc.tensor.matmul(
                                ps[:], a8_sb[:, 2 * p:2 * p + 2, ts(c, 128)],
                                xt[:, 2 * p:2 * p + 2, :],
                                start=(j == 0 and p == 0),
                                stop=(j == 1 and p == 3), perf_mode=DR)
                    nc.scalar.activation(xf[:, c, :], ps[:], Tanh,
                                         scale=2.0 ** -C_L[3])
                    i = n_pmc[0]
                    n_pmc[0] += 1
                    nc.tensor.matmul(pacc[0:B, :], m16_sb[:, off + c, :],
                                     xf[:, c, :],
                                     start=(i == 0), stop=(i == total_pmc - 1),
                                     skip_group_check=True)

                # next slot's head rides the l3 phase as PE filler; two
                # groups go first to cover the xw3 conversion chain
                nh = None
                if g + 1 < NG:
                    nh = make_head(g + 1, *slot_xs[g + 1], int(offs[g + 1]))
                    heads[g + 1] = nh
                    nh["x0t_group"](0)
                    nh["x0t_group"](1)
                for c in range(K):
                    l3_tile(c)
                    if nh is not None and c + 2 < 4:
                        nh["x0t_group"](c + 2)
                if nh is not None:
                    for i in range(max(min(K + 2, 4), 2), 4):
                        nh["x0t_group"](i)
            nc.vector.tensor_copy(out_sb[:], pacc[0:B, :])
            nc.sync.dma_start(out[:, :], out_sb[:])

    nc.compile()
    return nc


def _get_nc(NG, Ks):
    key = (NG, tuple(Ks), AMODE)
    if key not in _CACHE:
        _CACHE[key] = _build_nc(NG, Ks)
    return _CACHE[key]


def _prepare_in_maps(cdfg_xs, cdfg_as, graph, coverpoint_mask,
                     W_in, b_in, W_gcn, b_gcn):
    import ml_dtypes
    E4 = ml_dtypes.float8_e4m3
    E5 = ml_dtypes.float8_e5m2

    cdfg_xs = np.asarray(cdfg_xs, dtype=np.float32)
    cdfg_as = np.asarray(cdfg_as, dtype=np.float32)
    graph = np.asarray(graph).astype(np.int64)
    maskf = np.asarray(coverpoint_mask).astype(np.float32)
    W_in = np.asarray(W_in, dtype=np.float32)
    b_in = np.asarray(b_in, dtype=np.float32)
    W_gcn = np.asarray(W_gcn, dtype=np.float32)
    b_gcn = np.asarray(b_gcn, dtype=np.float32)
    assert not np.any(b_in) and not np.any(b_gcn), \
        "nonzero biases not supported by this build"

    need_a16 = False  # f16 A-mult reads the exact fp8 A (binary)

    uniq = np.unique(graph)
    u = len(uniq)
    NG = max(1, (u + NCORES - 1) // NCORES)

    perms, kts = {}, {}
    for gid in uniq:
        um = maskf[graph == gid].any(axis=0)
        perms[int(gid)] = np.argsort(~um, kind="stable")
        kts[int(gid)] = max(1, int(np.ceil(um.sum() / 128)))

    order = sorted(uniq.tolist(), key=lambda g: -kts[int(g)])
    Kb = []
    for b in range(NG):
        bucket = [kts[int(order[r])] for r in range(b * 8, min((b + 1) * 8, u))]
        Kb.append(max(bucket) if bucket else 1)
    # program slot s runs bucket perm[s]; the largest-K bucket goes last so
    # its many l3 tiles pipeline the exposed tail of the program
    perm = list(range(NG))
    Ks = [Kb[perm[s]] for s in range(NG)]
    T = sum(Ks)
    offs = np.concatenate([[0], np.cumsum(Ks)]).astype(int)

    # ---- weights (lifted fp16 + fp8 corrections)
    def f16r(x):
        return x.astype(np.float16).astype(np.float32)

    win16 = (f16r(W_in) * 2.0 ** C_IN).astype(np.float16)          # [F, H]
    dwin = ((W_in - f16r(W_in)) * 2.0 ** (S_XS + C_IN)).astype(E5)
    dwin8 = np.ascontiguousarray(
        np.broadcast_to(dwin[:, None, :], (F, 2, H))).astype(E5)
    wg16 = np.empty((128, 2 * L, H), np.float16)
    dwg8 = np.empty((128, 2 * L, H), E5)
    for l in range(L):
        wl16 = f16r(W_gcn[l])
        lift = (wl16 * 2.0 ** C_L[l]).astype(np.float16)           # [H, H]
        dw = ((W_gcn[l] - wl16)
              * 2.0 ** (S_X[min(l, 2)] + C_L[l])).astype(E5)
        for t in range(2):
            wg16[:, 2 * l + t, :] = lift[128 * t:128 * (t + 1), :]
            dwg8[:, 2 * l + t, :] = dw[128 * t:128 * (t + 1), :]

    common = {
        "win16_t": np.ascontiguousarray(win16),
        "dwin8_t": dwin8,
        "wg16_t": np.ascontiguousarray(wg16),
        "dwg8_t": np.ascontiguousarray(dwg8),
    }

    a16_cache, a8_cache, xs_cache = {}, {}, {}

    def graph_data(gid):
        if gid not in a8_cache:
            p = perms[gid]
            at = np.ascontiguousarray(cdfg_as[gid][p][:, p].T)
            if need_a16:
                a16_cache[gid] = at.astype(np.float16)
            a8_cache[gid] = at.astype(E4)
            xst = np.ascontiguousarray(cdfg_xs[gid][p].T)
            xs_cache[gid] = (xst.astype(np.float16),
                             (xst * 2.0 ** -S_XS).astype(E4))
        return gid

    in_maps = []
    for k in range(NCORES):
        a16_t = (np.empty((NG, N, N), np.float16) if need_a16 else None)
        a8_t = np.empty((NG, N, N), E4)
        xs16_t = np.empty((F, NG, N), np.float16)
        xs8d_t = np.zeros((F, NG, 2, N), E4)
        m16_t = np.zeros((128, T, B), np.float16)
        for s in range(NG):
            r = perm[s] * 8 + k
            gid = int(order[r]) if r < u else int(order[0])
            graph_data(gid)
            if need_a16:
                a16_t[s] = a16_cache[gid]
            a8_t[s] = a8_cache[gid]
            xs16_t[:, s, :] = xs_cache[gid][0]
            xs8d_t[:, s, 0, :] = xs_cache[gid][1]
            if r < u:
                p = perms[gid]
                rows = np.nonzero(graph == gid)[0]
                for bi in rows:
                    mp = (maskf[bi][p] / maskf[bi].sum()).astype(np.float16)
                    for c in range(kts[gid]):
                        m16_t[:, offs[s] + c, bi] = mp[c * 128:(c + 1) * 128]
        im = {"a8_t": a8_t, "xs16_t": xs16_t, "xs8d_t": xs8d_t,
              "m16_t": m16_t, **common}
        if need_a16:
            im["a16_t"] = a16_t
        in_maps.append(im)
    meta = {"NG": NG, "Ks": Ks, "order": order, "u": u}
    return in_maps, meta


def _assemble_out(results, graph, meta):
    graph = np.asarray(graph).astype(np.int64)
    out = np.zeros((B, H), dtype=np.float32)
    for r in range(meta["u"]):
        s, k = r // 8, r % 8
        rows = graph == meta["order"][r]
        out[rows] = results[k]["out"][rows]
    return out


def kernel(cdfg_xs, cdfg_as, graph, coverpoint_mask, W_in, b_in, W_gcn, b_gcn):
    from concourse.bass_utils import run_bass_kernel_spmd

    in_maps, meta = _prepare_in_maps(
        cdfg_xs, cdfg_as, graph, coverpoint_mask, W_in, b_in, W_gcn, b_gcn)
    nc = _get_nc(meta["NG"], meta["Ks"])
    res = run_bass_kernel_spmd(nc, in_maps, core_ids=list(range(NCORES)))
    return _assemble_out(res.results, graph, meta)


# revision 25
# speedup vs baseline: 1.0143x; 1.0143x over previous
"""Trainium2 Bass kernel for CdfgReader GNN message passing (fp16/fp8).

Strategy:
  - 64 batch items draw from <=32 unique CDFGs: compute the GNN once per
    unique graph; ceil(u/8) graph slots per core across 8 cores (SPMD).
  - Numerics (tolerance 2e-2, ~4e-3 measured end-to-end):
    * Main datapath fp16: activations x, A (binary, exact in any fp dtype),
      XW operands.  PE cost for fp16 == f32r (1 cycle/row) but DMA/SBUF
      halve and the vector engines work on half the bytes.
    * Weight systematic rounding error (the error-budget driver) is killed
      by a cheap fp8 DoubleRow correction matmul per X@W:
      XW = x16 @ fp16(W)*2^c + x8 @ e5m2((W - fp16(W)) * 2^(s+c)), where
      x8 = e4m3(x / 2^s).  DR contracts 2x128 rows at 0.5 cycles/row so the
      correction costs 1/4 of a full lo-pass.
    * Per-layer power-of-2 "lifts" 2^c are folded into the weights so every
      fp8 tensor sits in e4m3/e5m2 range; the relu/tanh ACT descales for
      free via its scale operand.
    * A-multiply per hidden layer is configurable: "f16" (dense fp16), "2t"
      (XW split hi+mid e4m3, DR at 0.5 cyc/row, 2x fewer PE cycles), "3t"
      (hi+mid+lo, ~11 effective bits, 1.33x fewer cycles).  The final layer
      always runs hi+lo DR fp8, restricted to the K 128-node tiles holding
      the coverpoint-mask union (host permutes nodes so the union is first).
    * XW psums are evicted to SBUF fp16 by one DVE copy; the fp8 splits
      derive from that copy so PSUM banks recycle at PE pace.
  - Masked mean via small fp16 mask matmuls (columns carry 1/count)
    accumulated in a dedicated PSUM bank across all slots.
"""

import numpy as np

NCORES = 8
N = 1024        # max nodes
F = 128         # input feature dim
H = 256         # hidden dim
L = 4           # GCN layers
B = 64          # batch (coverpoints)

# per-layer A-mult mode for hidden layers 0..2: "f16" | "2t" | "3t"
AMODE = ("f16", "f16", "2t")

C_IN = 4                 # lift on W_in
C_L = (5, 3, 1, -1)      # lift on W_gcn[l]
S_X = (0, 0, 5)          # x0/x1/x2 fp8 copy scales (x2's feeds XW3 corr too)
S_XS = 2                 # xs fp8 copy scale

_CACHE = {}


def _build_nc(NG, Ks):
    import concourse.bass as bass  # noqa: F401
    import concourse.mybir as mybir
    import concourse.tile as tile
    from concourse import bacc
    from concourse.bass import ts

    f32 = mybir.dt.float32
    f16 = mybir.dt.float16
    e4 = mybir.dt.float8e4
    e5 = mybir.dt.float8e5
    DR = mybir.MatmulPerfMode.DoubleRow
    Relu = mybir.ActivationFunctionType.Relu
    Tanh = mybir.ActivationFunctionType.Tanh
    Copy = mybir.ActivationFunctionType.Copy
    subop = mybir.AluOpType.subtract

    T = sum(Ks)
    offs = np.concatenate([[0], np.cumsum(Ks)]).astype(int)
    Kmax = max(Ks)
    need_a16 = False  # f16 A-mult reads the exact fp8 A (binary)

    nc = bacc.Bacc("TRN2", target_bir_lowering=False, debug=False,
                   num_devices=NCORES)

    a16_t = (nc.dram_tensor("a16_t", [NG, N, N], f16, kind="ExternalInput")
             if need_a16 else None)
    a8_t = nc.dram_tensor("a8_t", [NG, N, N], e4, kind="ExternalInput")
    xs16_t = nc.dram_tensor("xs16_t", [F, NG, N], f16, kind="ExternalInput")
    xs8d_t = nc.dram_tensor("xs8d_t", [F, NG, 2, N], e4, kind="ExternalInput")
    m16_t = nc.dram_tensor("m16_t", [128, T, B], f16, kind="ExternalInput")
    win16_t = nc.dram_tensor("win16_t", [128, H], f16, kind="ExternalInput")
    dwin8_t = nc.dram_tensor("dwin8_t", [128, 2, H], e5, kind="ExternalInput")
    wg16_t = nc.dram_tensor("wg16_t", [128, 2 * L, H], f16,
                            kind="ExternalInput")
    dwg8_t = nc.dram_tensor("dwg8_t", [128, 2 * L, H], e5,
                            kind="ExternalInput")
    out = nc.dram_tensor("out", [B, H], f32, kind="ExternalOutput")

    with tile.TileContext(nc) as tc:
        with (
            tc.tile_pool(name="const", bufs=1) as constp,
            tc.tile_pool(name="a16p", bufs=2) as a16p,
            tc.tile_pool(name="a8p", bufs=2) as a8p,
            tc.tile_pool(name="xsp", bufs=2) as xsp_pool,
            tc.tile_pool(name="xs8p", bufs=2) as xs8p,
            tc.tile_pool(name="x0p", bufs=2) as x0p,
            tc.tile_pool(name="x08p", bufs=2) as x08p,
            tc.tile_pool(name="x0np", bufs=2) as x0np,
            tc.tile_pool(name="xp", bufs=2) as xp,
            tc.tile_pool(name="x8p", bufs=2) as x8pool,
            tc.tile_pool(name="xwp", bufs=2) as xwp,
            tc.tile_pool(name="xw8p", bufs=2) as xw8p,
            tc.tile_pool(name="xfp", bufs=2) as xfp,
            tc.tile_pool(name="psx", bufs=3, space="PSUM") as psx,
            tc.tile_pool(name="psw", bufs=3, space="PSUM") as psw,
            tc.tile_pool(name="pacc", bufs=1, space="PSUM") as paccp,
        ):
            # ---- DMA priority: input-layer operands first
            xsp0 = xsp_pool.tile([128, 2, N], f16, tag="xs", name="xs_p")
            nc.sync.dma_start(xsp0[:, 0, 0:512], xs16_t[:, 0, 0:512])
            win16_sb = constp.tile([128, H], f16)
            nc.sync.dma_start(win16_sb[:], win16_t[:, :])
            dwin8_sb = constp.tile([128, 2, H], e5)
            nc.sync.dma_start(dwin8_sb[:], dwin8_t[:, :, :])
            xs8d0 = xs8p.tile([128, 2, N], e4, tag="xs8", name="xs8_p")
            nc.sync.dma_start(xs8d0[:], xs8d_t[:, 0, :, :])
            nc.sync.dma_start(xsp0[:, 0, 512:1024], xs16_t[:, 0, 512:1024])
            if NG > 1:
                nc.sync.dma_start(xsp0[:, 1, :], xs16_t[:, 1, :])
            xs8d1 = None
            if NG > 1:
                xs8d1 = xs8p.tile([128, 2, N], e4, tag="xs8", name="xs8_p")
                nc.sync.dma_start(xs8d1[:], xs8d_t[:, 1, :, :])

            wg16_sb = constp.tile([128, 2 * L, H], f16)
            dwg8_sb = constp.tile([128, 2 * L, H], e5)
            # layer-0 weights first so slot 0's layer 0 starts at DMA pace
            nc.sync.dma_start(wg16_sb[:, 0:2, :], wg16_t[:, 0:2, :])
            nc.sync.dma_start(dwg8_sb[:, 0:2, :], dwg8_t[:, 0:2, :])

            a16_sb0 = None
            if need_a16:
                a16_sb0 = a16p.tile([128, 8, N], f16, tag="a16", name="a16_sb")
                for q in range(4):
                    nc.sync.dma_start(
                        a16_sb0[:, 2 * q:2 * q + 2, :],
                        a16_t[0, 256 * q:256 * q + 256, :]
                        .rearrange("(mo p) i -> p mo i", p=128))
            a8_sb0 = a8p.tile([128, 8, N], e4, tag="a8", name="a8_sb")
            for q in range(4):
                nc.sync.dma_start(
                    a8_sb0[:, 2 * q:2 * q + 2, :],
                    a8_t[0, 256 * q:256 * q + 256, :]
                    .rearrange("(mo p) i -> p mo i", p=128))

            nc.sync.dma_start(wg16_sb[:, 2:8, :], wg16_t[:, 2:8, :])
            nc.sync.dma_start(dwg8_sb[:, 2:8, :], dwg8_t[:, 2:8, :])
            m16_sb = constp.tile([128, T, B], f16)
            nc.sync.dma_start(m16_sb[:], m16_t[:, :, :])

            out_sb = constp.tile([B, H], f32)
            pacc = paccp.tile([128, H], f32, tag="pacc", name="pacc")
            n_pmc = [0]
            total_pmc = T

            def make_head(g, xs16_g, xs8d_g, off_g):
                x0t = x0p.tile([128, 2, N], f16, tag="x0", name="x0t")
                x08 = x08p.tile([128, 2, N], e4, tag="x08", name="x08")
                x0n = x0np.tile([128, Kmax, H], f16, tag="x0n", name="x0n")

                def x0t_group(i):
                    t, c = [(0, 0), (1, 0), (0, 1), (1, 1)][i]
                    ps = psx.tile([128, 512], f32, tag="psx", name="ps0")
                    nc.tensor.matmul(ps[:], win16_sb[:, ts(t, 128)],
                                     xs16_g[:, ts(c, 512)],
                                     start=True, stop=False)
                    nc.tensor.matmul(ps[:], dwin8_sb[:, :, ts(t, 128)],
                                     xs8d_g[:, :, ts(c, 512)],
                                     start=False, stop=True, perf_mode=DR)
                    nc.scalar.activation(x0t[:, t, ts(c, 512)], ps[:],
                                         Relu, scale=2.0 ** -C_IN)
                    # s_x0 == 0: plain dtype-cast copy on Pool
                    nc.gpsimd.tensor_copy(x08[:, t, ts(c, 512)],
                                          x0t[:, t, ts(c, 512)])

                def x0n_group(c):
                    ps = psw.tile([128, H], f32, tag="ps3", name="ps0n",
                                  bufs=2)
                    nc.tensor.matmul(ps[:], xs16_g[:, ts(c, 128)],
                                     win16_sb[:], start=True, stop=False)
                    nc.tensor.matmul(ps[:], xs8d_g[:, :, ts(c, 128)],
                                     dwin8_sb[:], start=False, stop=True,
                                     perf_mode=DR)
                    nc.scalar.activation(x0n[:, c, :], ps[:], Relu,
                                         scale=2.0 ** -C_IN)

                return {"x0t": x0t, "x08": x08, "x0n": x0n,
                        "x0t_group": x0t_group, "x0n_group": x0n_group}

            heads = {}
            slot_xs = {0: (xsp0[:, 0, :], xs8d0)}
            if NG > 1:
                slot_xs[1] = (xsp0[:, 1, :], xs8d1)

            for g in range(NG):
                K = Ks[g]
                off = int(offs[g])
                if g == 0:
                    a16_sb, a8_sb = a16_sb0, a8_sb0
                else:
                    a16_sb, a8_sb = slot_a[g]
                xs_g, xs8d_g = slot_xs[g]

                # prefetch next slot's tensors now (pools are double
                # buffered) so its head can run as l3-phase filler
                if g + 1 < NG:
                    gn = g + 1
                    if gn >= 2:
                        xsn = xsp_pool.tile([128, 2, N], f16, tag="xs",
                                            name="xs_p")
                        nc.sync.dma_start(xsn[:, 0, :], xs16_t[:, gn, :])
                        xs8dn = xs8p.tile([128, 2, N], e4, tag="xs8",
                                          name="xs8_p")
                        nc.sync.dma_start(xs8dn[:], xs8d_t[:, gn, :, :])
                        slot_xs[gn] = (xsn[:, 0, :], xs8dn)
                    a16_n = None
                    if need_a16:
                        a16_n = a16p.tile([128, 8, N], f16, tag="a16",
                                          name="a16_sb")
                        nc.sync.dma_start(
                            a16_n[:],
                            a16_t[gn].rearrange("(mo p) i -> p mo i", p=128))
                    a8_n = a8p.tile([128, 8, N], e4, tag="a8", name="a8_sb")
                    nc.sync.dma_start(
                        a8_n[:],
                        a8_t[gn].rearrange("(mo p) i -> p mo i", p=128))
                    if g == 0:
                        slot_a = {}
                    slot_a[gn] = (a16_n, a8_n)

                cs = list(range(K))
                if g in heads:
                    # x0t groups were emitted as l3 filler in the previous
                    # slot; all x0n groups remain as boundary fillers
                    hd = heads.pop(g)
                    fillq = [lambda c=c: hd["x0n_group"](c) for c in cs]
                else:
                    hd = make_head(g, xs_g, xs8d_g, off)
                    for i in range(4):
                        hd["x0t_group"](i)
                    nfill = min(3, K - 1)
                    for c in cs[:K - nfill]:
                        hd["x0n_group"](c)
                    fillq = [lambda c=c: hd["x0n_group"](c)
                             for c in cs[K - nfill:]]
                x0t, x08, x0n = hd["x0t"], hd["x08"], hd["x0n"]
                x0n_group = hd["x0n_group"]

                def fill():
                    if fillq:
                        fillq.pop(0)()

                def mk_stage(idx, x16, x8):
                    # idx 0..2: hidden-layer XW; idx 3: final XW3 (hi/lo)
                    mode = "xw3" if idx == 3 else AMODE[idx]
                    st = {"mode": mode}
                    if mode == "f16":
                        st["r16"] = xwp.tile([128, 8, H], f16, tag="xw",
                                             name="r16")
                    else:
                        st["xwh"] = xw8p.tile([128, 8, H], e4, tag="xwh",
                                              name="xwh")
                        st["xwm"] = xw8p.tile([128, 8, H], e4, tag="xwm",
                                              name="xwm")
                        if mode == "3t":
                            st["xwl"] = xw8p.tile([128, 8, H], e4, tag="xwl",
                                                  name="xwl")
                            st["r2"] = xwp.tile([128, 8, H], f16, tag="xw",
                                                name="r2")

                    def pair(p, idx=idx, mode=mode, st=st, x16=x16, x8=x8):
                        ps = psw.tile([128, 2, H], f32, tag="psw",
                                      name="psw", bufs=2)
                        for i in range(2):
                            ch = 2 * p + i
                            nc.tensor.matmul(ps[:, i, :],
                                             x16[:, 0, ts(ch, 128)],
                                             wg16_sb[:, 2 * idx, :],
                                             start=True, stop=False)
                            nc.tensor.matmul(ps[:, i, :],
                                             x16[:, 1, ts(ch, 128)],
                                             wg16_sb[:, 2 * idx + 1, :],
                                             start=False, stop=False)
                            nc.tensor.matmul(
                                ps[:, i, :], x8[:, :, ts(ch, 128)],
                                dwg8_sb[:, 2 * idx:2 * idx + 2, :],
                                start=False, stop=True, perf_mode=DR)
                        sl = np.s_[:, 2 * p:2 * p + 2, :]
                        if mode == "f16":
                            nc.vector.tensor_copy(st["r16"][sl], ps[:])
                        elif mode in ("2t", "xw3"):
                            nc.scalar.activation(st["xwh"][sl], ps[:], Copy)
                            nc.vector.tensor_tensor(st["xwm"][sl], ps[:],
                                                    st["xwh"][sl], subop)
                        else:
                            nc.scalar.activation(st["xwh"][sl], ps[:], Copy)
                            nc.vector.tensor_tensor(st["r2"][sl], ps[:],
                                                    st["xwh"][sl], subop)
                            nc.gpsimd.tensor_copy(st["xwm"][sl],
                                                  st["r2"][sl])
                            nc.vector.tensor_tensor(st["xwl"][sl],
                                                    st["r2"][sl],
                                                    st["xwm"][sl], subop)

                    st["pair"] = pair
                    return st

                st = mk_stage(0, x0t, x08)
                st["pair"](0)
                fill()
                st["pair"](1)
                st["pair"](2)
                fill()
                st["pair"](3)

                for layer in range(L - 1):
                    mode = AMODE[layer]
                    if layer > 0:
                        st["pair"](2)
                        fill()
                        st["pair"](3)

                    pssA = [psx.tile([128, 512], f32, tag="psx",
                                     name=f"psA{t_}") for t_ in range(2)]
                    pssB = [psx.tile([128, 512], f32, tag="psx",
                                     name=f"psB{t_}") for t_ in range(2)]

                    def a16_pass(pss, c, m, st=st, a8_sb=a8_sb):
                        for t in range(2):
                            nc.tensor.matmul(
                                pss[t][:], st["r16"][:, m, ts(t, 128)],
                                a8_sb[:, m, ts(c, 512)],
                                start=(m == 0), stop=(m == 7))

                    def a8_sweep(pss, c, term, first, last, a8_sb=a8_sb,
                                 prange=(0, 4)):
                        for p in range(*prange):
                            for t in range(2):
                                nc.tensor.matmul(
                                    pss[t][:],
                                    term[:, 2 * p:2 * p + 2, ts(t, 128)],
                                    a8_sb[:, 2 * p:2 * p + 2, ts(c, 512)],
                                    start=(first and p == 0),
                                    stop=(last and p == 3),
                                    perf_mode=DR)

                    xn16 = xp.tile([128, 2, N], f16, tag="xn", name="xn16")
                    xn8 = x8pool.tile([128, 2, N], e4, tag="xn8", name="xn8")
                    s_next = S_X[min(layer + 1, 2)]

                    def act_half(pss, c, layer=layer, xn16=xn16, xn8=xn8,
                                 s_next=s_next):
                        nc.scalar.activation(xn16[:, 0, ts(c, 512)],
                                             pss[0][:], Relu,
                                             scale=2.0 ** -C_L[layer])
                        nc.vector.tensor_scalar(xn16[:, 1, ts(c, 512)],
                                                pss[1][:],
                                                2.0 ** -C_L[layer], 0.0,
                                                mybir.AluOpType.mult,
                                                mybir.AluOpType.max)
                        for t in range(2):
                            if s_next == 0:
                                nc.gpsimd.tensor_copy(
                                    xn8[:, t, ts(c, 512)],
                                    xn16[:, t, ts(c, 512)])
                            else:
                                nc.gpsimd.tensor_scalar_mul(
                                    xn8[:, t, ts(c, 512)],
                                    xn16[:, t, ts(c, 512)],
                                    2.0 ** -s_next)

                    # software pipeline: the next stage's first two XW pairs
                    # (which only need the c0 half) are emitted between the
                    # two act halves so every engine chain is PE-covered
                    nidx = layer + 1 if layer < 2 else 3
                    if mode == "f16":
                        for m in range(4):
                            a16_pass(pssA, 0, m)
                        fill()
                        for m in range(4):
                            a16_pass(pssB, 1, m)
                        for m in range(4, 8):
                            a16_pass(pssA, 0, m)
                        act_half(pssA, 0)
                        for m in range(4, 8):
                            a16_pass(pssB, 1, m)
                        nst = mk_stage(nidx, xn16, xn8)
                        nst["pair"](0)
                        nst["pair"](1)
                        act_half(pssB, 1)
                        fill()
                    else:
                        terms = ([st["xwh"], st["xwm"]] if mode == "2t"
                                 else [st["xwh"], st["xwm"], st["xwl"]])
                        nt = len(terms)
                        # first term's sweeps consume pairs 0-1 on both
                        # halves before touching pairs 2-3, buying the late
                        # pairs' ACT/DVE conversions more time
                        a8_sweep(pssA, 0, terms[0], True, nt == 1,
                                 prange=(0, 2))
                        fill()
                        a8_sweep(pssB, 1, terms[0], True, nt == 1,
                                 prange=(0, 2))
                        a8_sweep(pssA, 0, terms[0], False, nt == 1,
                                 prange=(2, 4))
                        fill()
                        a8_sweep(pssB, 1, terms[0], False, nt == 1,
                                 prange=(2, 4))
                        for j, term in enumerate(terms[1:-1], start=1):
                            a8_sweep(pssA, 0, term, False, False)
                            fill()
                            a8_sweep(pssB, 1, term, False, False)
                        if nt > 1:
                            a8_sweep(pssA, 0, terms[-1], False, True)
                        act_half(pssA, 0)
                        if nt > 1:
                            a8_sweep(pssB, 1, terms[-1], False, True)
                        nst = mk_stage(nidx, xn16, xn8)
                        nst["pair"](0)
                        nst["pair"](1)
                        act_half(pssB, 1)
                        fill()
                    st = nst

                # ---- final layer: DR hi/lo A-mult on the K masked tiles
                st["pair"](2)
                while fillq:
                    fillq.pop(0)()
                st["pair"](3)
                xw3h, xw3l = st["xwh"], st["xwm"]

                xf = xfp.tile([128, Kmax, H], f16, tag="xf", name="xf")

                def l3_tile(c, a8_sb=a8_sb, x0n=x0n, off=off):
                    ps = psw.tile([128, H], f32, tag="ps3", name="ps3",
                                  bufs=2)
                    for j, xt in enumerate((xw3h, xw3l)):
                        for p in range(4):
                            nc.tensor.matmul(
                                ps[:], a8_sb[:, 2 * p:2 * p + 2, ts(c, 128)],
                                xt[:, 2 * p:2 * p + 2, :],
                                start=(j == 0 and p == 0),
                                stop=(j == 1 and p == 3), perf_mode=DR)
                    nc.scalar.activation(xf[:, c, :], ps[:], Tanh,
                                         scale=2.0 ** -C_L[3])
                    nc.vector.tensor_add(xf[:, c, :], xf[:, c, :],
                                         x0n[:, c, :])
                    i = n_pmc[0]
                    n_pmc[0] += 1
                    nc.tensor.matmul(pacc[0:B, :], m16_sb[:, off + c, :],
                                     xf[:, c, :],
                                     start=(i == 0), stop=(i == total_pmc - 1),
                                     skip_group_check=True)

                # next slot's head rides the l3 phase as PE filler; two
                # groups go first to cover the xw3 conversion chain
                nh = None
                if g + 1 < NG:
                    nh = make_head(g + 1, *slot_xs[g + 1], int(offs[g + 1]))
                    heads[g + 1] = nh
                    nh["x0t_group"](0)
                    nh["x0t_group"](1)
                for c in range(K):
                    l3_tile(c)
                    if nh is not None and c + 2 < 4:
                        nh["x0t_group"](c + 2)
                if nh is not None:
                    for i in range(max(min(K + 2, 4), 2), 4):
                        nh["x0t_group"](i)
            nc.vector.tensor_copy(out_sb[:], pacc[0:B, :])
            nc.sync.dma_start(out[:, :], out_sb[:])

    nc.compile()
    return nc


def _get_nc(NG, Ks):
    key = (NG, tuple(Ks), AMODE)
    if key not in _CACHE:
        _CACHE[key] = _build_nc(NG, Ks)
    return _CACHE[key]


def _prepare_in_maps(cdfg_xs, cdfg_as, graph, coverpoint_mask,
                     W_in, b_in, W_gcn, b_gcn):
    import ml_dtypes
    E4 = ml_dtypes.float8_e4m3
    E5 = ml_dtypes.float8_e5m2

    cdfg_xs = np.asarray(cdfg_xs, dtype=np.float32)
    cdfg_as = np.asarray(cdfg_as, dtype=np.float32)
    graph = np.asarray(graph).astype(np.int64)
    maskf = np.asarray(coverpoint_mask).astype(np.float32)
    W_in = np.asarray(W_in, dtype=np.float32)
    b_in = np.asarray(b_in, dtype=np.float32)
    W_gcn = np.asarray(W_gcn, dtype=np.float32)
    b_gcn = np.asarray(b_gcn, dtype=np.float32)
    assert not np.any(b_in) and not np.any(b_gcn), \
        "nonzero biases not supported by this build"

    need_a16 = False  # f16 A-mult reads the exact fp8 A (binary)

    uniq = np.unique(graph)
    u = len(uniq)
    NG = max(1, (u + NCORES - 1) // NCORES)

    perms, kts = {}, {}
    for gid in uniq:
        um = maskf[graph == gid].any(axis=0)
        perms[int(gid)] = np.argsort(~um, kind="stable")
        kts[int(gid)] = max(1, int(np.ceil(um.sum() / 128)))

    order = sorted(uniq.tolist(), key=lambda g: -kts[int(g)])
    Kb = []
    for b in range(NG):
        bucket = [kts[int(order[r])] for r in range(b * 8, min((b + 1) * 8, u))]
        Kb.append(max(bucket) if bucket else 1)
    # program slot s runs bucket perm[s]; the largest-K bucket goes last so
    # its many l3 tiles pipeline the exposed tail of the program
    perm = list(range(NG))
    Ks = [Kb[perm[s]] for s in range(NG)]
    T = sum(Ks)
    offs = np.concatenate([[0], np.cumsum(Ks)]).astype(int)

    # ---- weights (lifted fp16 + fp8 corrections)
    def f16r(x):
        return x.astype(np.float16).astype(np.float32)

    win16 = (f16r(W_in) * 2.0 ** C_IN).astype(np.float16)          # [F, H]
    dwin = ((W_in - f16r(W_in)) * 2.0 ** (S_XS + C_IN)).astype(E5)
    dwin8 = np.ascontiguousarray(
        np.broadcast_to(dwin[:, None, :], (F, 2, H))).astype(E5)
    wg16 = np.empty((128, 2 * L, H), np.float16)
    dwg8 = np.empty((128, 2 * L, H), E5)
    for l in range(L):
        wl16 = f16r(W_gcn[l])
        lift = (wl16 * 2.0 ** C_L[l]).astype(np.float16)           # [H, H]
        dw = ((W_gcn[l] - wl16)
              * 2.0 ** (S_X[min(l, 2)] + C_L[l])).astype(E5)
        for t in range(2):
            wg16[:, 2 * l + t, :] = lift[128 * t:128 * (t + 1), :]
            dwg8[:, 2 * l + t, :] = dw[128 * t:128 * (t + 1), :]

    common = {
        "win16_t": np.ascontiguousarray(win16),
        "dwin8_t": dwin8,
        "wg16_t": np.ascontiguousarray(wg16),
        "dwg8_t": np.ascontiguousarray(dwg8),
    }

    a16_cache, a8_cache, xs_cache = {}, {}, {}

    def graph_data(gid):
        if gid not in a8_cache:
            p = perms[gid]
            at = np.ascontiguousarray(cdfg_as[gid][p][:, p].T)
            if need_a16:
                a16_cache[gid] = at.astype(np.float16)
            a8_cache[gid] = at.astype(E4)
            xst = np.ascontiguousarray(cdfg_xs[gid][p].T)
            xs_cache[gid] = (xst.astype(np.float16),
                             (xst * 2.0 ** -S_XS).astype(E4))
        return gid

    in_maps = []
    for k in range(NCORES):
        a16_t = (np.empty((NG, N, N), np.float16) if need_a16 else None)
        a8_t = np.empty((NG, N, N), E4)
        xs16_t = np.empty((F, NG, N), np.float16)
        xs8d_t = np.zeros((F, NG, 2, N), E4)
        m16_t = np.zeros((128, T, B), np.float16)
        for s in range(NG):
            r = perm[s] * 8 + k
            gid = int(order[r]) if r < u else int(order[0])
            graph_data(gid)
            if need_a16:
                a16_t[s] = a16_cache[gid]
            a8_t[s] = a8_cache[gid]
            xs16_t[:, s, :] = xs_cache[gid][0]
            xs8d_t[:, s, 0, :] = xs_cache[gid][1]
            if r < u:
                p = perms[gid]
                rows = np.nonzero(graph == gid)[0]
                for bi in rows:
                    mp = (maskf[bi][p] / maskf[bi].sum()).astype(np.float16)
                    for c in range(kts[gid]):
                        m16_t[:, offs[s] + c, bi] = mp[c * 128:(c + 1) * 128]
        im = {"a8_t": a8_t, "xs16_t": xs16_t, "xs8d_t": xs8d_t,
              "m16_t": m16_t, **common}
        if need_a16:
            im["a16_t"] = a16_t
        in_maps.append(im)
    meta = {"NG": NG, "Ks": Ks, "order": order, "u": u}
    return in_maps, meta


def _assemble_out(results, graph, meta):
    graph = np.asarray(graph).astype(np.int64)
    out = np.zeros((B, H), dtype=np.float32)
    for r in range(meta["u"]):
        s, k = r // 8, r % 8
        rows = graph == meta["order"][r]
        out[rows] = results[k]["out"][rows]
    return out


def kernel(cdfg_xs, cdfg_as, graph, coverpoint_mask, W_in, b_in, W_gcn, b_gcn):
    from concourse.bass_utils import run_bass_kernel_spmd

    in_maps, meta = _prepare_in_maps(
        cdfg_xs, cdfg_as, graph, coverpoint_mask, W_in, b_in, W_gcn, b_gcn)
    nc = _get_nc(meta["NG"], meta["Ks"])
    res = run_bass_kernel_spmd(nc, in_maps, core_ids=list(range(NCORES)))
    return _assemble_out(res.results, graph, meta)
